# revision 3
# baseline (speedup 1.0000x reference)
"""Per-sample 21x21 depthwise conv over (32, 3, 512, 512), 8-way data-parallel
on Trainium2 via Bass/Tile.

Algorithm: for each kernel column j, the H-direction 1D conv is a banded
(Toeplitz) matmul on the TensorEngine accumulated over j in PSUM:
out[h, w] = sum_{i,j} k[i,j] * xpad[h+i, w+j], chunked into 5 row-chunks of
M<=108 so the contraction window (M + 20 <= 128) fits the PE partition dim.

v3: fp8 e4m3 operands with perf_mode=DoubleRow, which processes TWO
weight/data pairs per PE cell per cycle (half the streaming time of
bf16/fp32r).  The two slots of each DoubleRow pair are used for PRECISION,
not extra contraction: slot0 = e4m3(k)-band, slot1 = e4m3(k - e4m3(k))-band,
while the moving operand carries the SAME x8 row in both slots, so each pair
computes (k8 + ek8) * x8 — the kernel quantization error is eliminated
(residual rms ~6e-4).  The x quantization error is shaped by 1D error
diffusion (sigma-delta) along W on the host: the 21x21 positive-mean kernel
is low-pass, so the high-frequency-shaped quantization noise largely cancels
in the 441-tap sum.  Exact offline check on the graded inputs:
max_rel_err = 1.53e-2 (tolerance 2e-2); bf16 fallback would be 2.3e-3.

All input tiles (60 chunk-tiles + 4 weight tiles) are DMA'd into SBUF once
up front; the per-rep body is a pure stream of 1260 DoubleRow matmuls +
PSUM->SBUF copies (alternating DVE/ACT) + output DMAs (alternating SP/Pool
queues), keeping the PE clock-gate warm.

Sharding: batch 32 -> 4 samples (12 images) per core, no communication.
"""
import sys

sys.path.insert(0, "/opt/trn_rl_repo")

import numpy as np
import ml_dtypes
import concourse.bacc as bacc
import concourse.tile as tile
import concourse.mybir as mybir
from concourse.bass import ds
from concourse.bass_utils import run_bass_kernel_spmd

B, C, H, W = 32, 3, 512, 512
KS, PAD = 21, 10
NCORES = 8
BPC = B // NCORES  # samples per core
T = BPC * C  # images per core
HP = H + 2 * PAD  # 532
WP = 544  # padded width 532 rounded up to 16B alignment for fp8 DoubleRow APs
MC = 108  # h_out rows per chunk (contraction window = MC + KS - 1 = 128)
MCA = 112  # band tile M stride (16B-aligned for DoubleRow lhsT AP)
CHUNKS = [(0, 108), (108, 108), (216, 108), (324, 108), (432, 80)]

FP8 = ml_dtypes.float8_e4m3

_nc_cache: dict = {}
_prep_cache: dict = {}


def _build_nc(reps: int = 1):
    fp8 = mybir.dt.float8e4
    f32 = mybir.dt.float32
    DR = mybir.MatmulPerfMode.DoubleRow
    S = BPC
    nc = bacc.Bacc(
        "TRN2", target_bir_lowering=False, debug=False, enable_asserts=False
    )
    xpad_d = nc.dram_tensor("xpad", [T, HP, WP], fp8, kind="ExternalInput")
    wb_d = nc.dram_tensor("wb", [S, 128, 2, KS, MCA], fp8, kind="ExternalInput")
    y_d = nc.dram_tensor("y", [T, H, W], f32, kind="ExternalOutput")

    with tile.TileContext(nc) as tc:
        with (
            tc.tile_pool(name="wp", bufs=1) as wp,
            tc.tile_pool(name="xp", bufs=1) as xp,
            tc.tile_pool(name="op", bufs=4) as op,
            tc.tile_pool(name="o2", bufs=4) as op2,
            tc.tile_pool(name="ps", bufs=6, space="PSUM") as psp,
        ):
            # hoisted input loads: persistent SBUF tiles for the whole rep loop
            wts = []
            for s in range(S):
                wt = wp.tile([128, 2, KS, MCA], fp8, tag=f"wt{s}")
                nc.sync.dma_start(
                    wt[:], wb_d[ds(s, 1)].rearrange("o p a k m -> (o p) a k m")
                )
                wts.append(wt)
            xts = {}
            for t in range(T):
                for ci, (c0, mc) in enumerate(CHUNKS):
                    kk = mc + KS - 1
                    xt = xp.tile([128, 2, WP], fp8, tag=f"xt{t}_{ci}")
                    # same image rows duplicated into both DoubleRow slots
                    for ko in range(2):
                        nc.sync.dma_start(
                            xt[0:kk, ko, :],
                            xpad_d[ds(t, 1), c0 : c0 + kk, :].rearrange(
                                "o p w -> (o p) w"
                            ),
                        )
                    xts[(t, ci)] = xt

            def rep_body():
                for t in range(T):
                    wt = wts[t // C]
                    for ci, (c0, mc) in enumerate(CHUNKS):
                        kk = mc + KS - 1
                        xt = xts[(t, ci)]
                        ps = psp.tile([128, 512], f32, tag="ps")
                        for j in range(KS):
                            nc.tensor.matmul(
                                ps[0:mc, :],
                                wt[0:kk, :, j, 0:mc],
                                xt[0:kk, :, j : j + W],
                                start=(j == 0),
                                stop=(j == KS - 1),
                                perf_mode=DR,
                            )
                        alt = (t * len(CHUNKS) + ci) % 2
                        pool = op if alt == 0 else op2
                        ot = pool.tile([128, 512], f32, tag="ot")
                        if alt == 0:
                            nc.vector.tensor_copy(ot[0:mc, :], ps[0:mc, :])
                        else:
                            nc.scalar.copy(ot[0:mc, :], ps[0:mc, :])
                        dma = nc.sync.dma_start if alt == 0 else nc.gpsimd.dma_start
                        dma(
                            y_d[ds(t, 1), c0 : c0 + mc, :].rearrange(
                                "o p w -> (o p) w"
                            ),
                            ot[0:mc, :],
                        )

            if reps == 1:
                rep_body()
            else:
                with tc.For_i(0, reps, 1, hint_engines=(mybir.EngineType.PE,)):
                    rep_body()

    nc.compile()
    return nc


def _host_prep(x: np.ndarray, kern: np.ndarray):
    """Error-diffused fp8 image; two-term (k8, ek8) banded Toeplitz weights."""
    xpad = np.zeros((B * C, HP, WP), np.float32)
    xpad[:, PAD : PAD + H, PAD : PAD + W] = x.reshape(B * C, H, W)

    # 1D sigma-delta along W: quantization error of column w is carried into
    # column w+1 before quantizing, pushing the noise spectrum high-frequency
    # where the low-pass conv kernel attenuates it.
    rows = xpad.reshape(-1, WP).astype(np.float64)
    x8 = np.zeros((rows.shape[0], WP), FP8)
    e = np.zeros(rows.shape[0])
    for w in range(WP):
        v = rows[:, w] + e
        q = v.astype(FP8)
        x8[:, w] = q
        e = v - q.astype(np.float64)
    x8 = x8.reshape(B * C, HP, WP)

    # two-term kernel split: k ~= k8 + ek8 exactly to ~6e-4 rms
    kf = kern[:, 0].astype(np.float64)  # (B, KS, KS)
    k8 = kf.astype(FP8)
    ek8 = (kf - k8.astype(np.float64)).astype(FP8)

    # band: Wb[s, p, ko, j, m] = kq[s, p - m, j] for 0 <= p - m < KS
    Wb = np.zeros((B, 128, 2, KS, MCA), FP8)
    m = np.arange(MC)
    i = np.arange(KS)
    P = np.broadcast_to(i[:, None, None] + m[None, None, :], (KS, KS, MC))
    J = np.broadcast_to(i[None, :, None], (KS, KS, MC))
    M = np.broadcast_to(m[None, None, :], (KS, KS, MC))
    I = np.broadcast_to(i[:, None, None], (KS, KS, MC))
    for ko, kq in ((0, k8), (1, ek8)):
        Wb[:, P, ko, J, M] = kq[:, I, J]
    return x8, Wb


def _execute(x: np.ndarray, kern: np.ndarray, reps: int = 1) -> np.ndarray:
    if reps not in _nc_cache:
        _nc_cache[reps] = _build_nc(reps)
    nc = _nc_cache[reps]
    key = (x.ctypes.data, kern.ctypes.data, x.shape)
    if key not in _prep_cache:
        _prep_cache.clear()
        _prep_cache[key] = _host_prep(np.asarray(x), np.asarray(kern))
    x8, Wb = _prep_cache[key]
    in_maps = [
        {
            "xpad": np.ascontiguousarray(x8[i * T : (i + 1) * T]),
            "wb": np.ascontiguousarray(Wb[i * BPC : (i + 1) * BPC]),
        }
        for i in range(NCORES)
    ]
    res = run_bass_kernel_spmd(nc, in_maps, list(range(NCORES)))
    y = np.concatenate([res.results[i]["y"] for i in range(NCORES)], axis=0)
    return y.reshape(B, C, H, W)


def kernel(x: np.ndarray, kernel: np.ndarray) -> np.ndarray:
    return _execute(x, kernel, reps=1)


# revision 4
# speedup vs baseline: 3.3016x; 3.3016x over previous
"""Per-sample 21x21 depthwise conv over (32, 3, 512, 512), 8-way data-parallel
on Trainium2 via Bass/Tile.

Algorithm: for each kernel column j, the H-direction 1D conv is a banded
(Toeplitz) matmul on the TensorEngine: out[h, w] = sum_{i,j} k[i,j] *
xpad[h+i, w+j].  Output rows are tiled in chunks of 108 so the contraction
window (108 + 20 = 128) fits the PE's 128-partition contraction dim exactly,
giving ONE matmul [K=128, M=108, N=512] per (chunk, j) accumulated in PSUM
over the 21 j's.  The band matrix B_j[p, m] = k[p-m, j] is translation
invariant, so one [128, 21, 108] SBUF tile per SAMPLE (channels share the
kernel) serves every chunk.

Performance structure (empirically tuned against this backend):
- operands in bf16: same PE streaming rate as float32r, half the DMA/SBUF.
- ALL input tiles (60 chunk-tiles + 4 weight tiles, ~9 MB bf16) are DMA'd
  into SBUF once up front; the steady-state body is a pure gap-free stream
  of 1260 matmuls, keeping the PE clock-gate (HAM) at full speed.
- output is written as bf16 (PSUM->SBUF copy downcasts; host upcasts to
  f32), halving output DMA traffic, and the 60 output DMAs are spread
  round-robin over THREE DMA queues (SP / Pool / Activation) with copies
  alternating DVE / ACT.  Without this the output path backpressures the
  PSUM-bank rotation and the PE drops out of its warm p-state (~290-310us);
  with it the kernel sits at the all-warm matmul floor (~233us, 184.8 ns/MM).
- PSUM pool of 6 banks (8 banks measured slower), output pools 2x4 bufs.

Numerics: bf16 operands give max rel err 2.2e-3; bf16 output rounding adds
~3e-3 -> ~5e-3 total, well inside the 2e-2 gate.  (An fp8 DoubleRow variant
with error-diffused inputs passes numerically at 1.52e-2 but this backend
prices matmuls by MOVING-OPERAND elements, so DoubleRow's doubled rhs free
size costs 2x, not 0.5x — rejected.)

Sharding: batch 32 -> 4 samples (12 images) per core, no communication.
"""
import sys

sys.path.insert(0, "/opt/trn_rl_repo")

import numpy as np
import ml_dtypes
import concourse.bacc as bacc
import concourse.tile as tile
import concourse.mybir as mybir
from concourse.bass import ds
from concourse.bass_utils import run_bass_kernel_spmd

B, C, H, W = 32, 3, 512, 512
KS, PAD = 21, 10
NCORES = 8
BPC = B // NCORES  # samples per core
T = BPC * C  # images per core
HP = WP = H + 2 * PAD  # 532
MC = 108  # h_out rows per chunk (contraction window = MC + KS - 1 = 128)
CHUNKS = [(0, 108), (108, 108), (216, 108), (324, 108), (432, 80)]

_nc_cache: dict = {}


def _build_nc(reps: int = 1):
    bf16 = mybir.dt.bfloat16
    f32 = mybir.dt.float32
    nc = bacc.Bacc(
        "TRN2", target_bir_lowering=False, debug=False, enable_asserts=False
    )
    xpad_d = nc.dram_tensor("xpad", [T, HP, WP], bf16, kind="ExternalInput")
    wb_d = nc.dram_tensor("wb", [BPC, 128, KS, MC], bf16, kind="ExternalInput")
    y_d = nc.dram_tensor("y", [T, H, W], bf16, kind="ExternalOutput")

    with tile.TileContext(nc) as tc:
        with (
            tc.tile_pool(name="wp", bufs=1) as wp,
            tc.tile_pool(name="xp", bufs=1) as xp,
            tc.tile_pool(name="op", bufs=4) as op,
            tc.tile_pool(name="o2", bufs=4) as op2,
            tc.tile_pool(name="ps", bufs=6, space="PSUM") as psp,
        ):
            # hoisted input loads: persistent SBUF tiles for the whole rep loop
            wts = []
            for s in range(BPC):
                wt = wp.tile([128, KS, MC], bf16, tag=f"wt{s}")
                nc.sync.dma_start(
                    wt[:], wb_d[ds(s, 1)].rearrange("o p k m -> (o p) k m")
                )
                wts.append(wt)
            xts = {}
            for t in range(T):
                for ci, (c0, mc) in enumerate(CHUNKS):
                    kk = mc + KS - 1
                    xt = xp.tile([128, WP], bf16, tag=f"xt{t}_{ci}")
                    nc.sync.dma_start(
                        xt[0:kk, :],
                        xpad_d[ds(t, 1), c0 : c0 + kk, :].rearrange(
                            "o p w -> (o p) w"
                        ),
                    )
                    xts[(t, ci)] = xt

            out_qs = [nc.sync.dma_start, nc.gpsimd.dma_start, nc.scalar.dma_start]

            def rep_body():
                idx = 0
                for t in range(T):
                    wt = wts[t // C]
                    for ci, (c0, mc) in enumerate(CHUNKS):
                        kk = mc + KS - 1
                        xt = xts[(t, ci)]
                        ps = psp.tile([128, 512], f32, tag="ps")
                        for j in range(KS):
                            nc.tensor.matmul(
                                ps[0:mc, :],
                                wt[0:kk, j, 0:mc],
                                xt[0:kk, j : j + W],
                                start=(j == 0),
                                stop=(j == KS - 1),
                            )
                        pool = op if idx % 2 == 0 else op2
                        ot = pool.tile([128, 512], bf16, tag="ot")
                        if idx % 2 == 0:
                            nc.vector.tensor_copy(ot[0:mc, :], ps[0:mc, :])
                        else:
                            nc.scalar.copy(ot[0:mc, :], ps[0:mc, :])
                        out_qs[idx % 3](
                            y_d[ds(t, 1), c0 : c0 + mc, :].rearrange(
                                "o p w -> (o p) w"
                            ),
                            ot[0:mc, :],
                        )
                        idx += 1

            if reps == 1:
                rep_body()
            else:
                with tc.For_i(0, reps, 1, hint_engines=(mybir.EngineType.PE,)):
                    rep_body()

    nc.compile()
    return nc


def _host_prep(x: np.ndarray, kern: np.ndarray):
    """Pad image (bf16); build per-sample banded Toeplitz weights (bf16)."""
    xpad = np.zeros((B, C, HP, WP), np.float32)
    xpad[:, :, PAD : PAD + H, PAD : PAD + W] = x

    # Wbs[s, p, j, m] = kern[s, 0, p - m, j] for 0 <= p - m < KS
    Wbs = np.zeros((B, 128, KS, MC), np.float32)
    m = np.arange(MC)
    i = np.arange(KS)
    P = np.broadcast_to(
        i[:, None, None] + m[None, None, :], (KS, KS, MC)
    )  # p = i + m
    J = np.broadcast_to(i[None, :, None], (KS, KS, MC))
    M = np.broadcast_to(m[None, None, :], (KS, KS, MC))
    I = np.broadcast_to(i[:, None, None], (KS, KS, MC))
    Wbs[:, P, J, M] = kern[:, 0][:, I, J]
    return (
        xpad.reshape(B * C, HP, WP).astype(ml_dtypes.bfloat16),
        Wbs.astype(ml_dtypes.bfloat16),
    )


def _execute(x: np.ndarray, kern: np.ndarray, reps: int = 1) -> np.ndarray:
    if reps not in _nc_cache:
        _nc_cache[reps] = _build_nc(reps)
    nc = _nc_cache[reps]
    xpad, Wbs = _host_prep(np.asarray(x), np.asarray(kern))
    in_maps = [
        {
            "xpad": np.ascontiguousarray(xpad[i * T : (i + 1) * T]),
            "wb": np.ascontiguousarray(Wbs[i * BPC : (i + 1) * BPC]),
        }
        for i in range(NCORES)
    ]
    res = run_bass_kernel_spmd(nc, in_maps, list(range(NCORES)))
    y = np.concatenate(
        [res.results[i]["y"].astype(np.float32) for i in range(NCORES)], axis=0
    )
    return y.reshape(B, C, H, W)


def kernel(x: np.ndarray, kernel: np.ndarray) -> np.ndarray:
    return _execute(x, kernel, reps=1)
